# revision 1
# baseline (speedup 1.0000x reference)
# BatchGAT Trainium2 Bass kernel — bucketed threshold-sum formulation.
#
# Reference computation (per batch b, head hd):
#   hp = h[b] @ w[hd]; t = tanh(hp)
#   s = t @ a_src[hd]; d = t @ a_dst[hd]
#   attn[i,j] = softmax_j(leaky_relu(s[i] + d[j], 0.2))
#   out = attn @ hp + bias_p
#
# Softmax_j is invariant to a per-i scale; multiplying by exp(-0.2 s_i)
# gives numerator terms max(e^{0.8 s_i} e^{d_j}, e^{0.2 d_j}) whose branch
# choice depends only on the ORDER of d_j vs -s_i. Quantizing d onto 127
# monotone buckets turns the n^2 attention sum into small bucket tables:
#   T1[k] = sum_{q(d_j)=k} e^{d_j} hp_ext[j],  T2[k] = sum e^{0.2 d_j} hp_ext[j]
#   num[i] = e^{0.8 s_i} * sum_{k>=t_i} T1[k] + Tot2 - sum_{k>=t_i} T2[k]
#   out[i] = num[i][:64] / num[i][64]        (hp_ext = [hp | 1], t_i = q(-s_i))
# T1/T2 ride one [128,130] table whose row 127 holds -Tot2 so a single
# step-mask matmul per i-tile yields [G1 | G2-Tot2]. All masks and the
# combine are batched n-wide DVE ops (stride-0 broadcast APs); instruction
# count per (b,h) pair is ~60 vs ~350 for the direct n^2 kernel.
#
# Sharding: head-parallel, one head per NeuronCore; each core does all 4
# batches of its head. h ships pre-transposed bf16 [b, 64, n]; bias_p is
# added on the host (out = attn@hp + b exactly).

import numpy as np
import ml_dtypes
from contextlib import ExitStack

import concourse.bass as bass
import concourse.tile as tile
import concourse.mybir as mybir
from concourse import bacc
from concourse.bass_utils import run_bass_kernel_spmd

F32 = mybir.dt.float32
BF16 = mybir.dt.bfloat16
I32 = mybir.dt.int32
AF = mybir.ActivationFunctionType
ALU = mybir.AluOpType

NB = 4      # batches
NF = 64     # f_in == f_out
NH = 8      # heads == cores
NBUCK = 128          # mask/table width; buckets 0..126, row 127 = -Tot2
KMAX = float(NBUCK - 2)
DLO, DHI = -5.0, 5.0
DELTA = (DHI - DLO) / NBUCK
RND = 8388608.0      # 2^23: x+RND-RND rounds f32 to nearest int
NW = 130             # combined table width: [T1(65) | T2(65)]


def _chunks(total, size):
    out = []
    c0 = 0
    while c0 < total:
        cs = min(size, total - c0)
        out.append((c0, cs))
        c0 += cs
    return out


def _rep0(ap_src, inner):
    # stride-0 inner broadcast: [128, m] -> [128, m, inner]
    return bass.AP(tensor=ap_src.tensor, offset=ap_src.offset,
                   ap=[list(ap_src.ap[0])] + [list(p) for p in ap_src.ap[1:]]
                   + [[0, inner]])


def build_gat_module(n=2048, nb=NB):
    nc = bacc.Bacc("TRN2", target_bir_lowering=False)

    ht_t = nc.dram_tensor("ht", [nb, NF, n], BF16, kind="ExternalInput")
    w_t = nc.dram_tensor("w1", [NF, NF], F32, kind="ExternalInput")
    asd_t = nc.dram_tensor("asd", [NF, 2], F32, kind="ExternalInput")
    o_t = nc.dram_tensor("out", [nb, n, NF], F32, kind="ExternalOutput")

    NT = n // 128
    C512 = _chunks(n, 512)
    nw = len(C512)
    WAVE = 6                     # gather wave size (psum banks: 6*256*4B = 3)

    with tile.TileContext(nc) as tc:
        with ExitStack() as ctx:
            consts = ctx.enter_context(tc.tile_pool(name="consts", bufs=1))
            hpool = ctx.enter_context(tc.tile_pool(name="hpool", bufs=1))
            work = ctx.enter_context(tc.tile_pool(name="work", bufs=6))
            pairbuf = ctx.enter_context(tc.tile_pool(name="pairbuf", bufs=2))
            outp = ctx.enter_context(tc.tile_pool(name="outp", bufs=2))
            pst = ctx.enter_context(tc.tile_pool(name="pst", bufs=1, space="PSUM"))
            pacc = ctx.enter_context(tc.tile_pool(name="pacc", bufs=1, space="PSUM"))
            drampool = ctx.enter_context(
                tc.tile_pool(name="drampool", bufs=2, space="DRAM"))

            # ---- constants ----
            from concourse.masks import make_identity
            ident_bf = consts.tile([128, 128], BF16)
            make_identity(nc, ident_bf)
            w_f32 = consts.tile([128, NF], F32)
            nc.sync.dma_start(out=w_f32[0:NF, :], in_=w_t[:, :])
            nc.sync.dma_start(out=w_f32[NF:128, :], in_=w_t[:, :])
            w_sb = consts.tile([128, NF], BF16)
            nc.vector.tensor_copy(w_sb, w_f32)
            asd_f32 = consts.tile([128, 2], F32)
            nc.sync.dma_start(out=asd_f32[0:NF, :], in_=asd_t[:, :])
            nc.sync.dma_start(out=asd_f32[NF:128, :], in_=asd_t[:, :])
            asd_sb = consts.tile([128, 2], BF16)
            nc.vector.tensor_copy(asd_sb, asd_f32)
            iota_i32 = consts.tile([128, NBUCK], I32)
            nc.gpsimd.iota(iota_i32, pattern=[[1, NBUCK]], base=0,
                           channel_multiplier=0)
            iota_row = consts.tile([128, NBUCK], BF16)
            nc.vector.tensor_copy(iota_row, iota_i32)
            iotac_i32 = consts.tile([128, 1], I32)
            nc.gpsimd.iota(iotac_i32, pattern=[[0, 1]], base=0,
                           channel_multiplier=1)
            iota_colf = consts.tile([128, 1], F32)
            nc.vector.tensor_copy(iota_colf, iotac_i32)
            negones = consts.tile([128, 1], BF16)
            nc.vector.memset(negones, -1.0)

            # ---- load pre-transposed h ----
            nhalf = nb // 2
            hTT = []
            for half in range(nhalf):
                row = []
                for q, (c0, cs) in enumerate(C512):
                    t_q = hpool.tile([128, 512], BF16, name=f"hTT{half}_{q}")
                    nc.sync.dma_start(
                        out=t_q[0:NF, 0:cs], in_=ht_t[2 * half, :, c0:c0 + cs])
                    nc.sync.dma_start(
                        out=t_q[NF:128, 0:cs],
                        in_=ht_t[2 * half + 1, :, c0:c0 + cs])
                    row.append(t_q)
                hTT.append(row)

            def stage1(b):
                half, bp = b // 2, NF * (b % 2)
                hTq = [hTT[half][q][bp:bp + NF, :] for q in range(nw)]
                w_b = w_sb[bp:bp + NF, :]
                st = {}

                # B: T = tanh(w.T @ hT) row layout [64, n]
                T_sb = pairbuf.tile([NF, n], BF16, name="T_sb")
                for icx, (c0, cs) in enumerate(C512):
                    psB = pst.tile([NF, 512], F32, name="psB")
                    mi = nc.tensor.matmul(
                        psB[:, 0:cs], lhsT=w_b, rhs=hTq[icx][:, 0:cs],
                        start=True, stop=True)
                    if icx > 0:
                        mi.ins.ldweights = False
                    nc.scalar.activation(
                        T_sb[:, c0:c0 + cs], psB[:, 0:cs], AF.Tanh)

                # D: s,d columns via psD[:, jb, 0|1]
                psD = pacc.tile([128, NT, 2], F32, name="psD")
                for jb in range(NT):
                    nc.tensor.matmul(
                        psD[:, jb, :],
                        lhsT=T_sb[:, jb * 128:(jb + 1) * 128],
                        rhs=asd_sb[0:NF, :], start=True, stop=True)

                # threshold bucket t_i = q(-s_i) first: col -> row via PE
                # transpose -> DRAM roundtrip broadcast (latency hidden
                # behind the rest of stage1 + F)
                qs = work.tile([128, NT], F32, name="qs")
                nc.scalar.mul(qs, psD[:, :, 0], -1.0 / DELTA)
                rs = work.tile([128, NT], F32, name="rs")
                nc.vector.tensor_scalar(
                    out=rs, in0=qs, scalar1=RND - DLO / DELTA, scalar2=RND,
                    op0=ALU.add, op1=ALU.subtract)
                bn_col = work.tile([128, NT], BF16, name="bn_col")
                nc.vector.tensor_scalar(
                    out=bn_col, in0=rs, scalar1=0.0, scalar2=KMAX,
                    op0=ALU.max, op1=ALU.min)
                psTr = pacc.tile([NT, 128], BF16, name="psTr")
                nc.tensor.transpose(psTr, bn_col, ident_bf)
                bn_row = work.tile([NT, 128], BF16, name="bn_row")
                nc.scalar.copy(bn_row, psTr)
                bn_dram = drampool.tile([NT, 128], BF16, name="bn_dram")
                nc.sync.dma_start(out=bn_dram, in_=bn_row)
                bn_bc = pairbuf.tile([128, n], BF16, name="bn_bc")
                bdap = bn_dram[0, 0:128]
                for (c0, cs) in C512:
                    nc.sync.dma_start(out=bn_bc[:, c0:c0 + cs], in_=bass.AP(
                        tensor=bdap.tensor, offset=bdap.offset + c0,
                        ap=[[0, 128], [1, cs]]))
                st["bn_bc"] = bn_bc

                e8s_col = pairbuf.tile([128, NT], F32, name="e8s_col")
                nc.scalar.activation(e8s_col, psD[:, :, 0], AF.Exp, scale=0.8)
                ed_col = pairbuf.tile([128, NT], BF16, name="ed_col")
                nc.scalar.activation(ed_col, psD[:, :, 1], AF.Exp)
                ed2_col = pairbuf.tile([128, NT], BF16, name="ed2_col")
                nc.scalar.activation(ed2_col, psD[:, :, 1], AF.Exp, scale=0.2)
                st["e8s_col"] = e8s_col

                # bucket(d_j) column: round+clip((d - DLO)/DELTA) to [0,126]
                qd = work.tile([128, NT], F32, name="qd")
                nc.scalar.mul(qd, psD[:, :, 1], 1.0 / DELTA)
                rd = work.tile([128, NT], F32, name="rd")
                nc.vector.tensor_scalar(
                    out=rd, in0=qd, scalar1=RND - DLO / DELTA, scalar2=RND,
                    op0=ALU.add, op1=ALU.subtract)
                kd_col = pairbuf.tile([128, NT], BF16, name="kd_col")
                nc.vector.tensor_scalar(
                    out=kd_col, in0=rd, scalar1=0.0, scalar2=KMAX,
                    op0=ALU.max, op1=ALU.min)

                # A: hp_ext[:, jb, 0:64] = hp rows, col 64 = 1.0
                hp_ext = pairbuf.tile([128, NT, 66], BF16, name="hp_ext")
                nc.vector.memset(hp_ext[:, :, 64:65], 1.0)
                for (j0, js) in _chunks(NT, 8):
                    psA = pst.tile([128, min(8, NT), NF], F32, name="psA")
                    for k in range(js):
                        jb = j0 + k
                        nc.tensor.matmul(
                            psA[:, k, :],
                            lhsT=hTq[jb // 4][:, (jb % 4) * 128:
                                              (jb % 4 + 1) * 128],
                            rhs=w_b, start=True, stop=True)
                    nc.scalar.copy(hp_ext[:, j0:j0 + js, 0:NF], psA[:, 0:js, :])

                # values: edhp_all = [ed*hp_ext | ed2*hp_ext]  (one tile)
                edhp_all = pairbuf.tile([128, NT, NW], BF16, name="edhp_all")
                nc.vector.tensor_tensor(
                    out=edhp_all[:, :, 0:65], in0=hp_ext[:, :, 0:65],
                    in1=_rep0(ed_col[:, :], 65), op=ALU.mult)
                nc.vector.tensor_tensor(
                    out=edhp_all[:, :, 65:130], in0=hp_ext[:, :, 0:65],
                    in1=_rep0(ed2_col[:, :], 65), op=ALU.mult)
                st["edhp_all"] = edhp_all

                # masks: onehot_all[j, jb, k] = (kd[j,jb] == k)
                onehot_all = pairbuf.tile([128, NT, NBUCK], BF16,
                                          name="onehot_all")
                iap = iota_row[:, :]
                nc.vector.tensor_tensor(
                    out=onehot_all, in0=_rep0(kd_col[:, :], NBUCK),
                    in1=bass.AP(tensor=iap.tensor, offset=iap.offset,
                                ap=[list(iap.ap[0]), [0, NT], [1, NBUCK]]),
                    op=ALU.is_equal)
                st["onehot_all"] = onehot_all
                return st

            def stageF(st):
                # scatter into combined table, then -Tot2 into row 127.
                # PE psum writes must start at partition 0/32/64, so -Tot2
                # lands in spare cols at partition 0 and a tiny SBUF->SBUF
                # DMA hops it across partitions into row 127.
                psT12 = pacc.tile([128, 196], F32, name="psT12")
                for jb in range(NT):
                    nc.tensor.matmul(
                        psT12[:, 0:NW], lhsT=st["onehot_all"][:, jb, :],
                        rhs=st["edhp_all"][:, jb, :],
                        start=(jb == 0), stop=(jb == NT - 1))
                T12_sb = pairbuf.tile([128, NW], BF16, name="T12_sb")
                nc.scalar.copy(T12_sb, psT12[:, 0:NW])
                nc.tensor.matmul(
                    psT12[0:1, 130:195], lhsT=negones[0:127, 0:1],
                    rhs=T12_sb[0:127, 65:130], start=True, stop=True,
                    skip_group_check=True)
                totrow = work.tile([1, 65], BF16, name="totrow")
                nc.scalar.copy(totrow, psT12[0:1, 130:195])
                nc.sync.dma_start(out=T12_sb[127:128, 65:130], in_=totrow)
                st["T12_sb"] = T12_sb
                return st

            def stageG(st, b):
                # step mask (deferred here so the bn_bc roundtrip has a full
                # stage1+F of emission to hide behind): hge[k,i] = (t_i <= k)
                hge_all = pairbuf.tile([128, n], BF16, name="hge_all")
                nc.vector.tensor_scalar(
                    out=hge_all, in0=st["bn_bc"], scalar1=iota_colf,
                    scalar2=None, op0=ALU.is_le)
                o_full = outp.tile([128, NT, NF], F32, name="o_full")
                for w0 in range(0, NT, WAVE):
                    ws = min(WAVE, NT - w0)
                    psG = pacc.tile([128, WAVE, 256], F32, name="psG")
                    for k in range(ws):
                        it = w0 + k
                        nc.tensor.matmul(
                            psG[:, k, 0:NW],
                            lhsT=hge_all[:, it * 128:(it + 1) * 128],
                            rhs=st["T12_sb"], start=True, stop=True)
                    # tmp = e8s*G1 ; numn = (G2-Tot2) - tmp = -num
                    tmp = work.tile([128, WAVE, 66], F32, name="tmp")
                    e8ap = st["e8s_col"][:, w0:w0 + ws]
                    nc.vector.tensor_tensor(
                        out=tmp[:, 0:ws, 0:65], in0=psG[:, 0:ws, 0:65],
                        in1=_rep0(e8ap, 65), op=ALU.mult)
                    numn = work.tile([128, WAVE, 66], F32, name="numn")
                    nc.vector.tensor_tensor(
                        out=numn[:, 0:ws, 0:65], in0=psG[:, 0:ws, 65:130],
                        in1=tmp[:, 0:ws, 0:65], op=ALU.subtract)
                    r = work.tile([128, WAVE], F32, name="r")
                    nc.vector.reciprocal(r[:, 0:ws], numn[:, 0:ws, 64:65])
                    # out = (-num)*(-1/den) on gpsimd (idle engine)
                    nc.gpsimd.tensor_tensor(
                        out=o_full[:, w0:w0 + ws, :], in0=numn[:, 0:ws, 0:64],
                        in1=_rep0(r[:, 0:ws], NF), op=ALU.mult)
                oap = o_t[b, :, :]
                nc.sync.dma_start(
                    out=bass.AP(tensor=oap.tensor, offset=oap.offset,
                                ap=[[NF, 128], [128 * NF, NT], [1, NF]]),
                    in_=o_full)

            prev = None
            for b in range(nb):
                st = stage1(b)
                if prev is not None:
                    stageG(prev[0], prev[1])
                stageF(st)
                prev = (st, b)
            stageG(prev[0], prev[1])

    nc.compile()
    return nc


_CACHE = {}
_last_results = None


def _get_nc(n=2048, nb=NB):
    key = (n, nb)
    if key not in _CACHE:
        _CACHE[key] = build_gat_module(n, nb)
    return _CACHE[key]


def kernel(h, adj, w, a_src, a_dst, bias_p):
    global _last_results
    h = np.asarray(h, dtype=np.float32)
    w = np.asarray(w, dtype=np.float32)
    a_src = np.asarray(a_src, dtype=np.float32)
    a_dst = np.asarray(a_dst, dtype=np.float32)
    bias_p = np.asarray(bias_p, dtype=np.float32)
    nb, n, _ = h.shape

    ht = np.ascontiguousarray(
        np.transpose(h, (0, 2, 1))).astype(ml_dtypes.bfloat16)

    nc = _get_nc(n, nb)
    in_maps = []
    for c in range(NH):
        asd = np.ascontiguousarray(
            np.concatenate([a_src[c], a_dst[c]], axis=1).astype(np.float32))
        in_maps.append({
            "ht": ht,
            "w1": np.ascontiguousarray(w[c]),
            "asd": asd,
        })
    res = run_bass_kernel_spmd(nc, in_maps, core_ids=list(range(NH)))
    _last_results = res
    out = np.empty((nb, NH, n, NF), np.float32)
    for c in range(NH):
        out[:, c] = res.results[c]["out"]
    # bias applied on host: out = attn@hp + bias (exact)
    out += bias_p[None, None, None, :]
    return out



# revision 7
# speedup vs baseline: 1.2464x; 1.2464x over previous
# BatchGAT Trainium2 Bass kernel — bucketed threshold-sum formulation,
# pair-fused edition.
#
# Reference computation (per batch b, head hd):
#   hp = h[b] @ w[hd]; t = tanh(hp)
#   s = t @ a_src[hd]; d = t @ a_dst[hd]
#   attn[i,j] = softmax_j(leaky_relu(s[i] + d[j], 0.2))
#   out = attn @ hp + bias_p
#
# Softmax_j is invariant to a per-i scale; multiplying by exp(-0.2 s_i)
# gives numerator terms max(e^{0.8 s_i} e^{d_j}, e^{0.2 d_j}) whose branch
# choice depends only on the ORDER of d_j vs -s_i. Quantizing d onto 127
# monotone buckets turns the n^2 attention sum into small bucket tables:
#   T1[k] = sum_{q(d_j)=k} e^{d_j} hp_ext[j],  T2[k] = sum e^{0.2 d_j} hp_ext[j]
#   num[i] = e^{0.8 s_i} * sum_{k>=t_i} T1[k] + Tot2 - sum_{k>=t_i} T2[k]
#   out[i] = num[i][:64] / num[i][64]        (hp_ext = [hp | 1], t_i = q(-s_i))
# T1/T2 ride one [128,130] table whose row 127 holds -Tot2 so a single
# step-mask matmul per i-tile yields [G1 | G2-Tot2].
#
# Pair-fusion: batches are processed two at a time with their feature dims
# stacked on the 128 partitions (rows 0:64 = even batch, 64:128 = odd), so
# every stage-1 matmul / activation / quantize op does two batches per
# instruction with the PE array fully loaded (K=128 instead of 64).
# A warmup burst of back-to-back matmuls runs under the input DMAs so the
# PE HAM clock gate is at 8/8 (2.4 GHz) before real work starts.
#
# Sharding: head-parallel, one head per NeuronCore; each core does all 4
# batches of its head. h ships pre-transposed bf16 [b, 64, n]; output is
# written bf16 in [128, NT*64] tile layout (host unscrambles + casts);
# bias_p is added on the host (out = attn@hp + b exactly).

import numpy as np
import ml_dtypes
from contextlib import ExitStack

import concourse.bass as bass
import concourse.tile as tile
import concourse.mybir as mybir
from concourse import bacc
from concourse.bass_utils import run_bass_kernel_spmd

F32 = mybir.dt.float32
BF16 = mybir.dt.bfloat16
I32 = mybir.dt.int32
AF = mybir.ActivationFunctionType
ALU = mybir.AluOpType

NB = 4      # batches
NF = 64     # f_in == f_out
NH = 8      # heads == cores
NBUCK = 128          # mask/table width; buckets 0..126, row 127 = -Tot2
KMAX = float(NBUCK - 2)
DLO, DHI = -5.0, 5.0
DELTA = (DHI - DLO) / NBUCK
RND = 8388608.0      # 2^23: x+RND-RND rounds f32 to nearest int
NW = 130             # combined table width: [T1(65) | T2(65)]
WAVE = 4             # stageG wave size (psum: 4*256*4B = 2 banks)
AWAVE = 4            # psA wave size


def _chunks(total, size):
    out = []
    c0 = 0
    while c0 < total:
        cs = min(size, total - c0)
        out.append((c0, cs))
        c0 += cs
    return out


def _rep0(ap_src, inner):
    # stride-0 inner broadcast: [...] -> [..., inner]
    return bass.AP(tensor=ap_src.tensor, offset=ap_src.offset,
                   ap=[list(p) for p in ap_src.ap] + [[0, inner]])


def _ap3(t, off, d1, d2):
    # build a 3D AP [128, d1, d2] over tile t at free-offset off with
    # explicit (stride, num) pairs d1, d2
    base = t[:, :] if len(t.shape) == 2 else t[:, :, :]
    return bass.AP(tensor=base.tensor, offset=base.offset + off,
                   ap=[list(base.ap[0]), list(d1), list(d2)])


def build_gat_module(n=2048, nb=NB):
    nc = bacc.Bacc("TRN2", target_bir_lowering=False)

    ht_t = nc.dram_tensor("ht", [nb, NF, n], BF16, kind="ExternalInput")
    w_t = nc.dram_tensor("w1", [NF, NF], F32, kind="ExternalInput")
    asd_t = nc.dram_tensor("asd", [NF, 2], F32, kind="ExternalInput")
    NT = n // 128
    o_t = nc.dram_tensor("out", [nb, 128, NT * NF], BF16, kind="ExternalOutput")

    C512 = _chunks(n, 512)
    nw = len(C512)
    npair = nb // 2

    with tile.TileContext(nc) as tc:
        with ExitStack() as ctx:
            consts = ctx.enter_context(tc.tile_pool(name="consts", bufs=1))
            hpool = ctx.enter_context(tc.tile_pool(name="hpool", bufs=1))
            work = ctx.enter_context(tc.tile_pool(name="work", bufs=2))
            pairbuf = ctx.enter_context(tc.tile_pool(name="pairbuf", bufs=2))
            outp = ctx.enter_context(tc.tile_pool(name="outp", bufs=2))
            pmm = ctx.enter_context(tc.tile_pool(name="pmm", bufs=2,
                                                 space="PSUM"))
            psm = ctx.enter_context(tc.tile_pool(name="psm", bufs=1,
                                                 space="PSUM"))
            pscat = ctx.enter_context(tc.tile_pool(name="pscat", bufs=1,
                                                   space="PSUM"))
            pG = ctx.enter_context(tc.tile_pool(name="pG", bufs=2,
                                                space="PSUM"))
            drampool = ctx.enter_context(
                tc.tile_pool(name="drampool", bufs=2, space="DRAM"))

            # ---- constants ----
            from concourse.masks import make_identity
            ident_bf = consts.tile([128, 128], BF16)
            make_identity(nc, ident_bf)

            # w_blk = block-diag(w, w) bf16 [128, 128]
            w_f32 = consts.tile([128, NF], F32)
            nc.sync.dma_start(out=w_f32[0:NF, :], in_=w_t[:, :])
            nc.sync.dma_start(out=w_f32[NF:128, :], in_=w_t[:, :])
            w_blk = consts.tile([128, 128], BF16)
            nc.vector.memset(w_blk, 0.0)
            nc.vector.tensor_copy(w_blk[0:NF, 0:NF], w_f32[0:NF, :])
            nc.vector.tensor_copy(w_blk[NF:128, NF:128], w_f32[NF:128, :])

            # asd_blk [128, 4]: cols 0,1 = (a_src|0, a_dst|0);
            # cols 2,3 = (0|a_src, 0|a_dst)
            asd_f32 = consts.tile([128, 2], F32)
            nc.sync.dma_start(out=asd_f32[0:NF, :], in_=asd_t[:, :])
            nc.sync.dma_start(out=asd_f32[NF:128, :], in_=asd_t[:, :])
            asd_blk = consts.tile([128, 4], BF16)
            nc.vector.memset(asd_blk, 0.0)
            nc.vector.tensor_copy(asd_blk[0:NF, 0:2], asd_f32[0:NF, :])
            nc.vector.tensor_copy(asd_blk[NF:128, 2:4], asd_f32[NF:128, :])

            iota_i32 = consts.tile([128, NBUCK], I32)
            nc.gpsimd.iota(iota_i32, pattern=[[1, NBUCK]], base=0,
                           channel_multiplier=0)
            iota_row = consts.tile([128, NBUCK], BF16)
            nc.vector.tensor_copy(iota_row, iota_i32)
            iotac_i32 = consts.tile([128, 1], I32)
            nc.gpsimd.iota(iotac_i32, pattern=[[0, 1]], base=0,
                           channel_multiplier=1)
            iota_colf = consts.tile([128, 1], F32)
            nc.vector.tensor_copy(iota_colf, iotac_i32)
            negones = consts.tile([128, 1], BF16)
            nc.vector.memset(negones, -1.0)

            # ---- PE warmup burst (runs under the input DMAs) ----
            # ~32 back-to-back N=128 matmuls ≈ 3.4µs of PE busy: pushes the
            # HAM clock gate to 8/8 before the real matmuls start.
            for i in range(32):
                pswu = pmm.tile([128, 128], F32, name="psmm")
                nc.tensor.matmul(pswu, lhsT=ident_bf, rhs=ident_bf,
                                 start=True, stop=True)

            # ---- load pre-transposed h: [128, 512] tiles, partitions
            # 0:64 = even batch features, 64:128 = odd batch ----
            hTT = []
            for p in range(npair):
                row = []
                for q, (c0, cs) in enumerate(C512):
                    t_q = hpool.tile([128, 512], BF16, name=f"hTT{p}_{q}")
                    nc.sync.dma_start(
                        out=t_q[0:NF, 0:cs], in_=ht_t[2 * p, :, c0:c0 + cs])
                    nc.sync.dma_start(
                        out=t_q[NF:128, 0:cs],
                        in_=ht_t[2 * p + 1, :, c0:c0 + cs])
                    row.append(t_q)
                hTT.append(row)

            def stage1(p):
                hTq = hTT[p]
                st = {}

                # B: T2 = tanh(w_blk.T @ hTT) [128, n] (both batches)
                T2_sb = pairbuf.tile([128, n], BF16, name="T2_sb")
                psD = psm.tile([128, NT, 4], F32, name="psD", tag="psdtr")
                for icx, (c0, cs) in enumerate(C512):
                    psB = pmm.tile([128, 512], F32, name="psmm")
                    # ldweights stays on: psD matmuls (different lhsT) are
                    # interleaved between B chunks in the PE stream
                    nc.tensor.matmul(
                        psB[:, 0:cs], lhsT=w_blk, rhs=hTq[icx][:, 0:cs],
                        start=True, stop=True)
                    nc.scalar.activation(
                        T2_sb[:, c0:c0 + cs], psB[:, 0:cs], AF.Tanh)
                    # D for the 4 i-tiles of this chunk:
                    # psD[:, jb, :] = [s_e, d_e, s_o, d_o] columns
                    for k in range(4):
                        jb = icx * 4 + k
                        nc.tensor.matmul(
                            psD[:, jb, :],
                            lhsT=T2_sb[:, jb * 128:(jb + 1) * 128],
                            rhs=asd_blk, start=True, stop=True)

                # strided views of psD: s cols {0,2}, d cols {1,3}
                s_sl = _ap3(psD, 0, [4, NT], [2, 2])
                d_sl = _ap3(psD, 1, [4, NT], [2, 2])

                # threshold bucket t_i = q(-s_i): col -> row via PE
                # transpose -> DRAM roundtrip broadcast (latency hidden
                # behind the rest of stage1)
                qs = work.tile([128, NT, 2], F32, name="qs")
                nc.scalar.mul(qs, s_sl, -1.0 / DELTA)
                rs = work.tile([128, NT, 2], F32, name="rs")
                nc.vector.tensor_scalar(
                    out=rs, in0=qs, scalar1=RND - DLO / DELTA, scalar2=RND,
                    op0=ALU.add, op1=ALU.subtract)
                # tr_in[:, 0:16] = even-batch buckets, [:, 16:32] = odd
                tr_in = work.tile([128, 32], BF16, name="tr_in")
                tr_out = _ap3(tr_in, 0, [1, NT], [NT, 2])
                nc.vector.tensor_scalar(
                    out=tr_out, in0=rs, scalar1=0.0, scalar2=KMAX,
                    op0=ALU.max, op1=ALU.min)
                psTr = psm.tile([32, 128], BF16, name="psTr", tag="psdtr")
                nc.tensor.transpose(psTr, tr_in, ident_bf)
                bn_row = work.tile([32, 128], BF16, name="bn_row")
                nc.scalar.copy(bn_row, psTr)
                bn_dram = drampool.tile([32, 128], BF16, name="bn_dram")
                nc.sync.dma_start(out=bn_dram, in_=bn_row)
                bdap = bn_dram[0, 0:128]
                for half, nm in ((0, "bn_bc_e"), (1, "bn_bc_o")):
                    bn_bc = pairbuf.tile([128, n], BF16, name=nm)
                    nc.sync.dma_start(out=bn_bc, in_=bass.AP(
                        tensor=bdap.tensor, offset=bdap.offset + half * n,
                        ap=[[0, 128], [1, n]]))
                    st[nm] = bn_bc

                # e8s / ed / ed2 columns (both batches at once)
                e8s2 = pairbuf.tile([128, NT, 2], F32, name="e8s2")
                nc.scalar.activation(e8s2, s_sl, AF.Exp, scale=0.8)
                ed2 = pairbuf.tile([128, NT, 2], BF16, name="ed2")
                nc.scalar.activation(ed2, d_sl, AF.Exp)
                ed22 = pairbuf.tile([128, NT, 2], BF16, name="ed22")
                nc.scalar.activation(ed22, d_sl, AF.Exp, scale=0.2)
                st["e8s2"] = e8s2

                # bucket(d_j): round+clip((d - DLO)/DELTA) to [0,126]
                qd = work.tile([128, NT, 2], F32, name="qd")
                nc.scalar.mul(qd, d_sl, 1.0 / DELTA)
                rd = work.tile([128, NT, 2], F32, name="rd")
                nc.vector.tensor_scalar(
                    out=rd, in0=qd, scalar1=RND - DLO / DELTA, scalar2=RND,
                    op0=ALU.add, op1=ALU.subtract)
                kd2 = pairbuf.tile([128, NT, 2], BF16, name="kd2")
                nc.vector.tensor_scalar(
                    out=kd2, in0=rd, scalar1=0.0, scalar2=KMAX,
                    op0=ALU.max, op1=ALU.min)

                # A: hp_ext2[:, jb, 0:65] = [hp_e | 1], [:, jb, 66:131] =
                # [hp_o | 1]
                hp_ext2 = pairbuf.tile([128, NT, 132], BF16, name="hp_ext2")
                ones_cols = _ap3(hp_ext2, NF, [132, NT], [NF + 2, 2])
                nc.vector.memset(ones_cols, 1.0)
                for (j0, js) in _chunks(NT, AWAVE):
                    psA = pmm.tile([128, AWAVE, 128], F32, name="psmm")
                    for k in range(js):
                        jb = j0 + k
                        nc.tensor.matmul(
                            psA[:, k, :],
                            lhsT=hTq[jb // 4][:, (jb % 4) * 128:
                                              (jb % 4 + 1) * 128],
                            rhs=w_blk, start=True, stop=True)
                    nc.scalar.copy(
                        hp_ext2[:, j0:j0 + js, 0:NF], psA[:, 0:js, 0:NF])
                    nc.scalar.copy(
                        hp_ext2[:, j0:j0 + js, NF + 2:NF * 2 + 2],
                        psA[:, 0:js, NF:128])

                # values: edhp_b = [ed*hp_ext | ed2*hp_ext] per batch
                for half, nm in ((0, "edhp_e"), (1, "edhp_o")):
                    edhp = pairbuf.tile([128, NT, NW], BF16, name=nm)
                    hpv = _ap3(hp_ext2, half * (NF + 2), [132, NT], [1, 65])
                    edc = _ap3(ed2, half, [2, NT], [0, 65])
                    ed2c = _ap3(ed22, half, [2, NT], [0, 65])
                    nc.vector.tensor_tensor(
                        out=edhp[:, :, 0:65], in0=hpv, in1=edc, op=ALU.mult)
                    nc.vector.tensor_tensor(
                        out=edhp[:, :, 65:130], in0=hpv, in1=ed2c,
                        op=ALU.mult)
                    st[nm] = edhp

                # masks: onehot_b[j, jb, k] = (kd[j,jb] == k) per batch
                iap = iota_row[:, :]
                irow = bass.AP(tensor=iap.tensor, offset=iap.offset,
                               ap=[list(iap.ap[0]), [0, NT], [1, NBUCK]])
                for half, nm in ((0, "onehot_e"), (1, "onehot_o")):
                    onehot = pairbuf.tile([128, NT, NBUCK], BF16, name=nm)
                    kdc = _ap3(kd2, half, [2, NT], [0, NBUCK])
                    nc.vector.tensor_tensor(
                        out=onehot, in0=kdc, in1=irow, op=ALU.is_equal)
                    st[nm] = onehot
                return st

            def stageF(st, half):
                # scatter into combined table, then -Tot2 into row 127.
                sfx = "_e" if half == 0 else "_o"
                onehot = st["onehot" + sfx]
                edhp = st["edhp" + sfx]
                psT12 = pscat.tile([128, 256], F32, name="psT12")
                for jb in range(NT):
                    nc.tensor.matmul(
                        psT12[:, 0:NW], lhsT=onehot[:, jb, :],
                        rhs=edhp[:, jb, :],
                        start=(jb == 0), stop=(jb == NT - 1))
                T12_sb = pairbuf.tile([128, NW], BF16, name="T12" + sfx)
                nc.scalar.copy(T12_sb, psT12[:, 0:NW])
                nc.tensor.matmul(
                    psT12[0:1, 130:195], lhsT=negones[0:127, 0:1],
                    rhs=T12_sb[0:127, 65:130], start=True, stop=True,
                    skip_group_check=True)
                totrow = work.tile([1, 65], BF16, name="totrow" + sfx)
                nc.scalar.copy(totrow, psT12[0:1, 130:195])
                nc.sync.dma_start(out=T12_sb[127:128, 65:130], in_=totrow)
                st["T12" + sfx] = T12_sb

            def stageG(st, p, half):
                sfx = "_e" if half == 0 else "_o"
                b = 2 * p + half
                bn_bc = st["bn_bc" + sfx]
                T12_sb = st["T12" + sfx]
                e8s2 = st["e8s2"]
                # step mask: hge[k,i] = (t_i <= k)
                hge = pairbuf.tile([128, n], BF16, name="hge" + sfx)
                nc.vector.tensor_scalar(
                    out=hge, in0=bn_bc, scalar1=iota_colf,
                    scalar2=None, op0=ALU.is_le)
                o_full = outp.tile([128, NT, NF], BF16, name="o_full" + sfx)
                for w0 in range(0, NT, WAVE):
                    ws = min(WAVE, NT - w0)
                    psG = pG.tile([128, WAVE, 256], F32, name="psG")
                    for k in range(ws):
                        it = w0 + k
                        nc.tensor.matmul(
                            psG[:, k, 0:NW],
                            lhsT=hge[:, it * 128:(it + 1) * 128],
                            rhs=T12_sb, start=True, stop=True)
                    # tmp = e8s*G1 ; numn = (G2-Tot2) - tmp = -num
                    tmp = work.tile([128, WAVE, 66], F32, name="tmp")
                    e8b = e8s2[:, :, :]
                    e8ap = bass.AP(
                        tensor=e8b.tensor,
                        offset=e8b.offset + w0 * 2 + half,
                        ap=[list(e8b.ap[0]), [2, ws], [0, 65]])
                    nc.vector.tensor_tensor(
                        out=tmp[:, 0:ws, 0:65], in0=psG[:, 0:ws, 0:65],
                        in1=e8ap, op=ALU.mult)
                    numn = work.tile([128, WAVE, 66], F32, name="numn")
                    nc.vector.tensor_tensor(
                        out=numn[:, 0:ws, 0:65], in0=psG[:, 0:ws, 65:130],
                        in1=tmp[:, 0:ws, 0:65], op=ALU.subtract)
                    r = work.tile([128, WAVE], F32, name="r")
                    nc.vector.reciprocal(r[:, 0:ws], numn[:, 0:ws, 64:65])
                    # out = (-num)*(-1/den) on gpsimd (idle engine)
                    nc.gpsimd.tensor_tensor(
                        out=o_full[:, w0:w0 + ws, :], in0=numn[:, 0:ws, 0:64],
                        in1=_rep0(r[:, 0:ws], NF), op=ALU.mult)
                oap = o_t[b, :, :]
                nc.sync.dma_start(
                    out=bass.AP(tensor=oap.tensor, offset=oap.offset,
                                ap=[[NT * NF, 128], [NF, NT], [1, NF]]),
                    in_=o_full)

            # software pipeline: s1(0) F(0) s1(1) G(0) F(1) G(1)
            st0 = stage1(0)
            stageF(st0, 0)
            stageF(st0, 1)
            st1 = stage1(1)
            stageG(st0, 0, 0)
            stageG(st0, 0, 1)
            stageF(st1, 0)
            stageF(st1, 1)
            stageG(st1, 1, 0)
            stageG(st1, 1, 1)

    nc.compile()
    return nc


_CACHE = {}
_last_results = None


def _get_nc(n=2048, nb=NB):
    key = (n, nb)
    if key not in _CACHE:
        _CACHE[key] = build_gat_module(n, nb)
    return _CACHE[key]


def kernel(h, adj, w, a_src, a_dst, bias_p):
    global _last_results
    h = np.asarray(h, dtype=np.float32)
    w = np.asarray(w, dtype=np.float32)
    a_src = np.asarray(a_src, dtype=np.float32)
    a_dst = np.asarray(a_dst, dtype=np.float32)
    bias_p = np.asarray(bias_p, dtype=np.float32)
    nb, n, _ = h.shape
    NT = n // 128

    ht = np.ascontiguousarray(
        np.transpose(h, (0, 2, 1))).astype(ml_dtypes.bfloat16)

    nc = _get_nc(n, nb)
    in_maps = []
    for c in range(NH):
        asd = np.ascontiguousarray(
            np.concatenate([a_src[c], a_dst[c]], axis=1).astype(np.float32))
        in_maps.append({
            "ht": ht,
            "w1": np.ascontiguousarray(w[c]),
            "asd": asd,
        })
    res = run_bass_kernel_spmd(nc, in_maps, core_ids=list(range(NH)))
    _last_results = res
    out = np.empty((nb, NH, n, NF), np.float32)
    for c in range(NH):
        # device layout [nb, 128, NT*NF] bf16 -> [nb, n, NF] f32
        dev = res.results[c]["out"].astype(np.float32)
        out[:, c] = dev.reshape(nb, 128, NT, NF).transpose(
            0, 2, 1, 3).reshape(nb, n, NF)
    # bias applied on host: out = attn@hp + bias (exact)
    out += bias_p[None, None, None, :]
    return out


# revision 10
# speedup vs baseline: 1.2743x; 1.0224x over previous
# BatchGAT Trainium2 Bass kernel — bucketed threshold-sum formulation,
# pair-fused + latency-optimized edition.
#
# Reference computation (per batch b, head hd):
#   hp = h[b] @ w[hd]; t = tanh(hp)
#   s = t @ a_src[hd]; d = t @ a_dst[hd]
#   attn[i,j] = softmax_j(leaky_relu(s[i] + d[j], 0.2))
#   out = attn @ hp + bias_p
#
# Softmax_j is invariant to a per-i scale; multiplying by exp(-0.2 s_i)
# gives numerator terms max(e^{0.8 s_i} e^{d_j}, e^{0.2 d_j}) whose branch
# choice depends only on the ORDER of d_j vs -s_i. Quantizing d onto 127
# monotone buckets turns the n^2 attention sum into small bucket tables:
#   T1[k] = sum_{q(d_j)=k} e^{d_j} hp_ext[j],  T2[k] = sum e^{0.2 d_j} hp_ext[j]
#   num[i] = e^{0.8 s_i} * sum_{k>=t_i} T1[k] + Tot2 - sum_{k>=t_i} T2[k]
#   out[i] = num[i][:64] / num[i][64]        (hp_ext = [hp | 1], t_i = q(-s_i))
# T1/T2 ride one [128,130] table whose row 127 holds -Tot2 so a single
# step-mask matmul per i-tile yields [G1 | G2-Tot2].
#
# Bucket ranges are ADAPTIVE: the host computes max|s|, max|d| per head
# (cheap BLAS) and pre-scales the a_src/a_dst columns by 1/DELTA, so the
# device gets bucket indices straight out of the s/d matmul (no separate
# scale ops) and the buckets are ~2x tighter than a fixed [-5,5] range.
# The s-side threshold skips rounding entirely (a sub-bucket boundary
# shift, same order as the quantization error itself).
#
# Pair-fusion: batches are processed two at a time with their feature dims
# stacked on the 128 partitions (rows 0:64 = even batch, 64:128 = odd), so
# every stage-1 matmul / activation / quantize op does two batches per
# instruction with the PE array fully loaded (K=128 instead of 64).
# A warmup burst of back-to-back matmuls runs under the input DMAs so the
# PE HAM clock gate is at 8/8 (2.4 GHz) before real work starts.
#
# Sharding: head-parallel, one head per NeuronCore; each core does all 4
# batches of its head. h ships pre-transposed bf16 [b, 64, n]; output is
# written bf16 in [128, NT*64] tile layout (host unscrambles + casts);
# bias_p is added on the host (out = attn@hp + b exactly).

import numpy as np
import ml_dtypes
from contextlib import ExitStack

import concourse.bass as bass
import concourse.tile as tile
import concourse.mybir as mybir
from concourse import bacc
from concourse.bass_utils import run_bass_kernel_spmd

F32 = mybir.dt.float32
BF16 = mybir.dt.bfloat16
I32 = mybir.dt.int32
AF = mybir.ActivationFunctionType
ALU = mybir.AluOpType

NB = 4      # batches
NF = 64     # f_in == f_out
NH = 8      # heads == cores
NBUCK = 128          # mask/table width; buckets 0..126, row 127 = -Tot2
KMAX = float(NBUCK - 2)
CMID = 63.0          # bucket center offset; host scales give |x| <= 62
RND = 8388608.0      # 2^23: x+RND-RND rounds f32 to nearest int
NW = 130             # combined table width: [T1(65) | T2(65)]
WAVE = 4             # stageG wave size (4 waves/batch, 2 psum tags)


def _chunks(total, size):
    out = []
    c0 = 0
    while c0 < total:
        cs = min(size, total - c0)
        out.append((c0, cs))
        c0 += cs
    return out


def _rep0(ap_src, inner):
    # stride-0 inner broadcast: [...] -> [..., inner]
    return bass.AP(tensor=ap_src.tensor, offset=ap_src.offset,
                   ap=[list(p) for p in ap_src.ap] + [[0, inner]])


def _apx(t, off, *dims):
    # AP [128, *dims] over tile t at free-offset off; dims are explicit
    # (stride, num) pairs
    base = t[tuple([slice(None)] * len(t.shape))]
    return bass.AP(tensor=base.tensor, offset=base.offset + off,
                   ap=[list(base.ap[0])] + [list(d) for d in dims])


def build_gat_module(n=2048, nb=NB):
    nc = bacc.Bacc("TRN2", target_bir_lowering=False)

    ht_t = nc.dram_tensor("ht", [nb, NF, n], BF16, kind="ExternalInput")
    w_t = nc.dram_tensor("w1", [NF, NF], F32, kind="ExternalInput")
    asd_t = nc.dram_tensor("asd", [NF, 4], F32, kind="ExternalInput")
    NT = n // 128
    o_t = nc.dram_tensor("out", [nb, 128, NT * NF], BF16, kind="ExternalOutput")

    C512 = _chunks(n, 512)
    npair = nb // 2

    with tile.TileContext(nc) as tc:
        with ExitStack() as ctx:
            consts = ctx.enter_context(tc.tile_pool(name="consts", bufs=1))
            hpool = ctx.enter_context(tc.tile_pool(name="hpool", bufs=1))
            work = ctx.enter_context(tc.tile_pool(name="work", bufs=2))
            pairbuf = ctx.enter_context(tc.tile_pool(name="pairbuf", bufs=2))
            outp = ctx.enter_context(tc.tile_pool(name="outp", bufs=2))
            pmm = ctx.enter_context(tc.tile_pool(name="pmm", bufs=2,
                                                 space="PSUM"))
            psm = ctx.enter_context(tc.tile_pool(name="psm", bufs=1,
                                                 space="PSUM"))
            pscat = ctx.enter_context(tc.tile_pool(name="pscat", bufs=1,
                                                   space="PSUM"))
            pGa = ctx.enter_context(tc.tile_pool(name="pGa", bufs=1,
                                                 space="PSUM"))
            pGb = ctx.enter_context(tc.tile_pool(name="pGb", bufs=1,
                                                 space="PSUM"))
            drampool = ctx.enter_context(
                tc.tile_pool(name="drampool", bufs=2, space="DRAM"))

            # ---- constants ----
            from concourse.masks import make_identity
            ident_bf = consts.tile([128, 128], BF16)
            make_identity(nc, ident_bf)

            # ---- PE warmup burst (runs under the input DMAs) ----
            for i in range(28):
                pswu = pmm.tile([128, 128], F32, name="psmm", tag="psmm")
                nc.tensor.matmul(pswu, lhsT=ident_bf, rhs=ident_bf,
                                 start=True, stop=True)

            # w_blk = block-diag(w, w) bf16 [128, 128]
            w_f32 = consts.tile([128, NF], F32)
            nc.sync.dma_start(out=w_f32[0:NF, :], in_=w_t[:, :])
            nc.sync.dma_start(out=w_f32[NF:128, :], in_=w_t[:, :])
            w_blk = consts.tile([128, 128], BF16)
            nc.vector.memset(w_blk, 0.0)
            nc.vector.tensor_copy(w_blk[0:NF, 0:NF], w_f32[0:NF, :])
            nc.vector.tensor_copy(w_blk[NF:128, NF:128], w_f32[NF:128, :])

            # asd_blk [128, 8]: rows 0:64 cols 0:4 = [-a_src/Ds, a_dst/Dd,
            # a_src, a_dst]; rows 64:128 cols 4:8 = same (odd batch)
            asd_f32 = consts.tile([128, 4], F32)
            nc.sync.dma_start(out=asd_f32[0:NF, :], in_=asd_t[:, :])
            nc.sync.dma_start(out=asd_f32[NF:128, :], in_=asd_t[:, :])
            asd_blk = consts.tile([128, 8], BF16)
            nc.vector.memset(asd_blk, 0.0)
            nc.vector.tensor_copy(asd_blk[0:NF, 0:4], asd_f32[0:NF, :])
            nc.vector.tensor_copy(asd_blk[NF:128, 4:8], asd_f32[NF:128, :])

            iota_i32 = consts.tile([128, NBUCK], I32)
            nc.gpsimd.iota(iota_i32, pattern=[[1, NBUCK]], base=0,
                           channel_multiplier=0)
            iota_row = consts.tile([128, NBUCK], BF16)
            nc.vector.tensor_copy(iota_row, iota_i32)
            iotac_i32 = consts.tile([128, 1], I32)
            nc.gpsimd.iota(iotac_i32, pattern=[[0, 1]], base=0,
                           channel_multiplier=1)
            iota_colf = consts.tile([128, 1], F32)
            nc.vector.tensor_copy(iota_colf, iotac_i32)
            negones = consts.tile([128, 1], BF16)
            nc.vector.memset(negones, -1.0)

            # ---- load pre-transposed h: one [128, n] tile per pair,
            # partitions 0:64 = even batch, 64:128 = odd ----
            hTT = []
            for p in range(npair):
                hT2 = hpool.tile([128, n], BF16, name=f"hT2_{p}")
                nc.sync.dma_start(out=hT2[0:NF, :], in_=ht_t[2 * p, :, :])
                nc.sync.dma_start(out=hT2[NF:128, :],
                                  in_=ht_t[2 * p + 1, :, :])
                hTT.append(hT2)

            def stage1(p):
                hT2 = hTT[p]
                st = {}

                # B: T2 = tanh(w_blk.T @ hT2) [128, n] (both batches)
                T2_sb = pairbuf.tile([128, n], BF16, name="T2_sb")
                psD = psm.tile([128, NT, 8], F32, name="psD", tag="psdtr")
                for icx, (c0, cs) in enumerate(C512):
                    psB = pmm.tile([128, 512], F32, name="psmm", tag="psmm")
                    nc.tensor.matmul(
                        psB[:, 0:cs], lhsT=w_blk, rhs=hT2[:, c0:c0 + cs],
                        start=True, stop=True)
                    nc.scalar.activation(
                        T2_sb[:, c0:c0 + cs], psB[:, 0:cs], AF.Tanh)
                    # D: psD[:, jb, :] = per-batch [x_s, x_d, s, d] columns
                    # (x_s = -s/Ds, x_d = d/Dd via host-prescaled asd cols)
                    for k in range(4):
                        jb = icx * 4 + k
                        nc.tensor.matmul(
                            psD[:, jb, :],
                            lhsT=T2_sb[:, jb * 128:(jb + 1) * 128],
                            rhs=asd_blk, start=True, stop=True)

                # threshold bucket bn_i = clip(x_s + CMID): col -> row via
                # PE transpose -> DRAM roundtrip broadcast (latency hidden
                # behind the rest of stage1). No rounding: sub-bucket
                # boundary shift only.
                # tr_in[:, 0:16] = even-batch buckets, [:, 16:32] = odd
                tr_in = work.tile([128, 32], BF16, name="tr_in")
                nc.vector.tensor_scalar(
                    out=_apx(tr_in, 0, [1, NT], [NT, 2]),
                    in0=_apx(psD, 0, [8, NT], [4, 2]),
                    scalar1=CMID, scalar2=KMAX, op0=ALU.add, op1=ALU.min)
                psTr = psm.tile([32, 128], BF16, name="psTr", tag="psdtr")
                nc.tensor.transpose(psTr, tr_in, ident_bf)
                bn_row = work.tile([32, 128], BF16, name="bn_row")
                nc.scalar.copy(bn_row, psTr)
                bn_dram = drampool.tile([32, 128], BF16, name="bn_dram")
                nc.sync.dma_start(out=bn_dram, in_=bn_row)
                bdap = bn_dram[0, 0:128]
                # both batches' broadcast rows land in one [128, 2, n] tile
                bn_bc = pairbuf.tile([128, 2, n], BF16, name="bn_bc")
                for half in range(2):
                    nc.sync.dma_start(out=bn_bc[:, half, :], in_=bass.AP(
                        tensor=bdap.tensor, offset=bdap.offset + half * n,
                        ap=[[0, 128], [1, n]]))
                st["bn_bc"] = bn_bc

                # e8s / ed / ed2 columns (both batches per op)
                s_raw = _apx(psD, 2, [8, NT], [4, 2])
                d_raw = _apx(psD, 3, [8, NT], [4, 2])
                e8s2 = pairbuf.tile([128, NT, 2], F32, name="e8s2")
                nc.scalar.activation(e8s2, s_raw, AF.Exp, scale=0.8)
                # edc2 [128, NT, 4]: cols (ed_e, ed2_e, ed_o, ed2_o)
                edc2 = pairbuf.tile([128, NT, 4], BF16, name="edc2")
                nc.scalar.activation(
                    _apx(edc2, 0, [4, NT], [2, 2]), d_raw, AF.Exp)
                nc.scalar.activation(
                    _apx(edc2, 1, [4, NT], [2, 2]), d_raw, AF.Exp, scale=0.2)
                st["e8s2"] = e8s2

                # bucket(d_j): round(x_d + CMID), clip to [0, KMAX]
                rd = work.tile([128, NT, 2], F32, name="rd")
                nc.vector.tensor_scalar(
                    out=rd, in0=_apx(psD, 1, [8, NT], [4, 2]),
                    scalar1=RND + CMID, scalar2=RND,
                    op0=ALU.add, op1=ALU.subtract)
                kd2 = pairbuf.tile([128, NT, 2], BF16, name="kd2")
                nc.vector.tensor_scalar(
                    out=kd2, in0=rd, scalar1=0.0, scalar2=KMAX,
                    op0=ALU.max, op1=ALU.min)

                # A: hp_ext2[:, jb, 0:65] = [hp_e | 1], [66:131] = [hp_o | 1]
                hp_ext2 = pairbuf.tile([128, NT, 132], BF16, name="hp_ext2")
                nc.vector.memset(_apx(hp_ext2, NF, [132, NT], [NF + 2, 2]),
                                 1.0)
                for (j0, js) in _chunks(NT, 4):
                    psA = pmm.tile([128, 4, 128], F32, name="psmm",
                                   tag="psmm")
                    for k in range(js):
                        jb = j0 + k
                        nc.tensor.matmul(
                            psA[:, k, :],
                            lhsT=hT2[:, jb * 128:(jb + 1) * 128],
                            rhs=w_blk, start=True, stop=True)
                    nc.scalar.copy(
                        hp_ext2[:, j0:j0 + js, 0:NF], psA[:, 0:js, 0:NF])
                    nc.scalar.copy(
                        hp_ext2[:, j0:j0 + js, NF + 2:NF * 2 + 2],
                        psA[:, 0:js, NF:128])

                # values: edhp_b = [ed*hp_ext | ed2*hp_ext], one op per batch
                for half, nm in ((0, "edhp_e"), (1, "edhp_o")):
                    edhp = pairbuf.tile([128, NT, NW], BF16, name=nm)
                    nc.vector.tensor_tensor(
                        out=_apx(edhp, 0, [NW, NT], [65, 2], [1, 65]),
                        in0=_apx(hp_ext2, half * (NF + 2),
                                 [132, NT], [0, 2], [1, 65]),
                        in1=_apx(edc2, half * 2, [4, NT], [1, 2], [0, 65]),
                        op=ALU.mult)
                    st[nm] = edhp

                # masks: onehot[j, jb, b, k] = (kd[j,jb,b] == k), one op
                onehot2 = pairbuf.tile([128, NT, 2, NBUCK], BF16,
                                       name="onehot2")
                iap = iota_row[:, :]
                nc.vector.tensor_tensor(
                    out=onehot2,
                    in0=_apx(kd2, 0, [2, NT], [1, 2], [0, NBUCK]),
                    in1=bass.AP(tensor=iap.tensor, offset=iap.offset,
                                ap=[list(iap.ap[0]), [0, NT], [0, 2],
                                    [1, NBUCK]]),
                    op=ALU.is_equal)
                st["onehot2"] = onehot2
                return st

            def stageF(st, half):
                # scatter into combined table, then -Tot2 into row 127.
                sfx = "_e" if half == 0 else "_o"
                onehot2 = st["onehot2"]
                edhp = st["edhp" + sfx]
                psT12 = pscat.tile([128, 256], F32, name="psT12")
                for jb in range(NT):
                    nc.tensor.matmul(
                        psT12[:, 0:NW], lhsT=onehot2[:, jb, half, :],
                        rhs=edhp[:, jb, :],
                        start=(jb == 0), stop=(jb == NT - 1))
                T12_sb = pairbuf.tile([128, NW], BF16, name="T12" + sfx)
                nc.scalar.copy(T12_sb, psT12[:, 0:NW])
                nc.tensor.matmul(
                    psT12[0:1, 130:195], lhsT=negones[0:127, 0:1],
                    rhs=T12_sb[0:127, 65:130], start=True, stop=True,
                    skip_group_check=True)
                totrow = work.tile([1, 65], BF16, name="totrow" + sfx)
                nc.scalar.copy(totrow, psT12[0:1, 130:195])
                nc.sync.dma_start(out=T12_sb[127:128, 65:130], in_=totrow)
                st["T12" + sfx] = T12_sb

            def stageG(st, p, half):
                sfx = "_e" if half == 0 else "_o"
                b = 2 * p + half
                bn_bc = st["bn_bc"]
                T12_sb = st["T12" + sfx]
                e8s2 = st["e8s2"]
                # step mask: hge[k,i] = (t_i <= k)
                hge = pairbuf.tile([128, n], BF16, name="hge" + sfx)
                nc.vector.tensor_scalar(
                    out=hge, in0=bn_bc[:, half, :], scalar1=iota_colf,
                    scalar2=None, op0=ALU.is_le)
                o_full = outp.tile([128, NT, NF], BF16, name="o_full" + sfx)
                # G-matmul waves -> scalar-engine copy to SBUF; the whole
                # batch then combines in 3 wide DVE ops instead of 12
                gsb = work.tile([128, NT, NW], F32, name="gsb")
                for wv, w0 in enumerate(range(0, NT, WAVE)):
                    ws = min(WAVE, NT - w0)
                    pool_w = pGa if wv % 2 == 0 else pGb
                    psG = pool_w.tile([128, WAVE, 256], F32,
                                      name=f"psG{'ab'[wv % 2]}")
                    for k in range(ws):
                        it = w0 + k
                        nc.tensor.matmul(
                            psG[:, k, 0:NW],
                            lhsT=hge[:, it * 128:(it + 1) * 128],
                            rhs=T12_sb, start=True, stop=True)
                    nc.scalar.copy(gsb[:, w0:w0 + ws, :],
                                   psG[:, 0:ws, 0:NW])
                # tmp = e8s*G1 ; numn = (G2-Tot2) - tmp = -num
                tmp = work.tile([128, NT, 65], F32, name="tmp")
                e8b = e8s2[:, :, :]
                e8ap = bass.AP(
                    tensor=e8b.tensor, offset=e8b.offset + half,
                    ap=[list(e8b.ap[0]), [2, NT], [0, 65]])
                nc.vector.tensor_tensor(
                    out=tmp, in0=_apx(gsb, 0, [NW, NT], [1, 65]),
                    in1=e8ap, op=ALU.mult)
                numn = work.tile([128, NT, 65], F32, name="numn")
                nc.vector.tensor_tensor(
                    out=numn, in0=_apx(gsb, 65, [NW, NT], [1, 65]),
                    in1=tmp, op=ALU.subtract)
                r = work.tile([128, NT], F32, name="r")
                nc.vector.reciprocal(r, numn[:, :, 64:65])
                # out = (-num)*(-1/den); alternate gpsimd / vector per batch
                eng = nc.gpsimd if b % 2 == 0 else nc.vector
                eng.tensor_tensor(
                    out=o_full, in0=numn[:, :, 0:64],
                    in1=_rep0(r, NF), op=ALU.mult)
                oap = o_t[b, :, :]
                nc.sync.dma_start(
                    out=bass.AP(tensor=oap.tensor, offset=oap.offset,
                                ap=[[NT * NF, 128], [NF, NT], [1, NF]]),
                    in_=o_full)

            # software pipeline: all scatters before all gathers so the PE
            # in-order stream never stalls on a roundtrip DMA
            st0 = stage1(0)
            stageF(st0, 0)
            stageF(st0, 1)
            st1 = stage1(1)
            stageF(st1, 0)
            stageF(st1, 1)
            stageG(st0, 0, 0)
            stageG(st0, 0, 1)
            stageG(st1, 1, 0)
            stageG(st1, 1, 1)

    nc.compile()
    return nc


_CACHE = {}
_last_results = None


def _get_nc(n=2048, nb=NB):
    key = (n, nb)
    if key not in _CACHE:
        _CACHE[key] = build_gat_module(n, nb)
    return _CACHE[key]


def kernel(h, adj, w, a_src, a_dst, bias_p):
    global _last_results
    h = np.asarray(h, dtype=np.float32)
    w = np.asarray(w, dtype=np.float32)
    a_src = np.asarray(a_src, dtype=np.float32)
    a_dst = np.asarray(a_dst, dtype=np.float32)
    bias_p = np.asarray(bias_p, dtype=np.float32)
    nb, n, _ = h.shape
    NT = n // 128

    ht = np.ascontiguousarray(
        np.transpose(h, (0, 2, 1))).astype(ml_dtypes.bfloat16)

    # adaptive bucket scales: max|s|, max|d| per head (BLAS, cheap)
    hf = h.reshape(-1, h.shape[-1])
    nc = _get_nc(n, nb)
    in_maps = []
    for c in range(NH):
        th = np.tanh(hf @ w[c])
        s = th @ a_src[c, :, 0]
        d = th @ a_dst[c, :, 0]
        ds = max(float(np.abs(s).max()), 1e-6) / 62.0
        dd = max(float(np.abs(d).max()), 1e-6) / 62.0
        asd = np.stack([-a_src[c, :, 0] / ds, a_dst[c, :, 0] / dd,
                        a_src[c, :, 0], a_dst[c, :, 0]],
                       axis=1).astype(np.float32)
        in_maps.append({
            "ht": ht,
            "w1": np.ascontiguousarray(w[c]),
            "asd": np.ascontiguousarray(asd),
        })
    res = run_bass_kernel_spmd(nc, in_maps, core_ids=list(range(NH)))
    _last_results = res
    out = np.empty((nb, NH, n, NF), np.float32)
    for c in range(NH):
        # device layout [nb, 128, NT*NF] bf16 -> [nb, n, NF] f32
        dev = res.results[c]["out"].astype(np.float32)
        out[:, c] = dev.reshape(nb, 128, NT, NF).transpose(
            0, 2, 1, 3).reshape(nb, n, NF)
    # bias applied on host: out = attn@hp + bias (exact)
    out += bias_p[None, None, None, :]
    return out


# revision 11
# speedup vs baseline: 1.2792x; 1.0038x over previous
# BatchGAT Trainium2 Bass kernel — bucketed threshold-sum formulation,
# pair-fused + latency-optimized edition.
#
# Reference computation (per batch b, head hd):
#   hp = h[b] @ w[hd]; t = tanh(hp)
#   s = t @ a_src[hd]; d = t @ a_dst[hd]
#   attn[i,j] = softmax_j(leaky_relu(s[i] + d[j], 0.2))
#   out = attn @ hp + bias_p
#
# Softmax_j is invariant to a per-i scale; multiplying by exp(-0.2 s_i)
# gives numerator terms max(e^{0.8 s_i} e^{d_j}, e^{0.2 d_j}) whose branch
# choice depends only on the ORDER of d_j vs -s_i. Quantizing d onto 127
# monotone buckets turns the n^2 attention sum into small bucket tables:
#   T1[k] = sum_{q(d_j)=k} e^{d_j} hp_ext[j],  T2[k] = sum e^{0.2 d_j} hp_ext[j]
#   num[i] = e^{0.8 s_i} * sum_{k>=t_i} T1[k] + Tot2 - sum_{k>=t_i} T2[k]
#   out[i] = num[i][:64] / num[i][64]        (hp_ext = [hp | 1], t_i = q(-s_i))
# T1/T2 ride one [128,130] table whose row 127 holds -Tot2 so a single
# step-mask matmul per i-tile yields [G1 | G2-Tot2].
#
# Bucket ranges are ADAPTIVE: the host computes max|s|, max|d| per head
# (cheap BLAS) and pre-scales the a_src/a_dst columns by 1/DELTA, so the
# device gets bucket indices straight out of the s/d matmul (no separate
# scale ops) and the buckets are ~2x tighter than a fixed [-5,5] range.
# The s-side threshold skips rounding entirely (a sub-bucket boundary
# shift, same order as the quantization error itself).
#
# Pair-fusion: batches are processed two at a time with their feature dims
# stacked on the 128 partitions (rows 0:64 = even batch, 64:128 = odd), so
# every stage-1 matmul / activation / quantize op does two batches per
# instruction with the PE array fully loaded (K=128 instead of 64).
# A warmup burst of back-to-back matmuls runs under the input DMAs so the
# PE HAM clock gate is at 8/8 (2.4 GHz) before real work starts.
#
# Sharding: head-parallel, one head per NeuronCore; each core does all 4
# batches of its head. h ships pre-transposed bf16 [b, 64, n]; output is
# written bf16 in [128, NT*64] tile layout (host unscrambles + casts);
# bias_p is added on the host (out = attn@hp + b exactly).

import numpy as np
import ml_dtypes
from contextlib import ExitStack

import concourse.bass as bass
import concourse.tile as tile
import concourse.mybir as mybir
from concourse import bacc
from concourse.bass_utils import run_bass_kernel_spmd

F32 = mybir.dt.float32
BF16 = mybir.dt.bfloat16
I32 = mybir.dt.int32
AF = mybir.ActivationFunctionType
ALU = mybir.AluOpType

NB = 4      # batches
NF = 64     # f_in == f_out
NH = 8      # heads == cores
NBUCK = 128          # mask/table width; buckets 0..126, row 127 = -Tot2
KMAX = float(NBUCK - 2)
CMID = 63.0          # bucket center offset; host scales give |x| <= 62
RND = 8388608.0      # 2^23: x+RND-RND rounds f32 to nearest int
NW = 130             # combined table width: [T1(65) | T2(65)]
WAVE = 4             # stageG wave size (4 waves/batch, 2 psum tags)


def _chunks(total, size):
    out = []
    c0 = 0
    while c0 < total:
        cs = min(size, total - c0)
        out.append((c0, cs))
        c0 += cs
    return out


def _rep0(ap_src, inner):
    # stride-0 inner broadcast: [...] -> [..., inner]
    return bass.AP(tensor=ap_src.tensor, offset=ap_src.offset,
                   ap=[list(p) for p in ap_src.ap] + [[0, inner]])


def _apx(t, off, *dims):
    # AP [128, *dims] over tile t at free-offset off; dims are explicit
    # (stride, num) pairs
    base = t[tuple([slice(None)] * len(t.shape))]
    return bass.AP(tensor=base.tensor, offset=base.offset + off,
                   ap=[list(base.ap[0])] + [list(d) for d in dims])


def build_gat_module(n=2048, nb=NB):
    nc = bacc.Bacc("TRN2", target_bir_lowering=False)

    ht_t = nc.dram_tensor("ht", [nb, NF, n], BF16, kind="ExternalInput")
    w_t = nc.dram_tensor("w1", [NF, NF], F32, kind="ExternalInput")
    asd_t = nc.dram_tensor("asd", [NF, 4], F32, kind="ExternalInput")
    NT = n // 128
    o_t = nc.dram_tensor("out", [nb, 128, NT * NF], BF16, kind="ExternalOutput")

    C512 = _chunks(n, 512)
    npair = nb // 2

    with tile.TileContext(nc) as tc:
        with ExitStack() as ctx:
            consts = ctx.enter_context(tc.tile_pool(name="consts", bufs=1))
            hpool = ctx.enter_context(tc.tile_pool(name="hpool", bufs=1))
            work = ctx.enter_context(tc.tile_pool(name="work", bufs=2))
            pairbuf = ctx.enter_context(tc.tile_pool(name="pairbuf", bufs=2))
            outp = ctx.enter_context(tc.tile_pool(name="outp", bufs=2))
            pmm = ctx.enter_context(tc.tile_pool(name="pmm", bufs=2,
                                                 space="PSUM"))
            psm = ctx.enter_context(tc.tile_pool(name="psm", bufs=1,
                                                 space="PSUM"))
            pscat = ctx.enter_context(tc.tile_pool(name="pscat", bufs=1,
                                                   space="PSUM"))
            pGa = ctx.enter_context(tc.tile_pool(name="pGa", bufs=1,
                                                 space="PSUM"))
            pGb = ctx.enter_context(tc.tile_pool(name="pGb", bufs=1,
                                                 space="PSUM"))
            drampool = ctx.enter_context(
                tc.tile_pool(name="drampool", bufs=2, space="DRAM"))

            # ---- constants ----
            from concourse.masks import make_identity
            ident_bf = consts.tile([128, 128], BF16)
            make_identity(nc, ident_bf)

            # ---- PE warmup burst (runs under the input DMAs) ----
            for i in range(28):
                pswu = pmm.tile([128, 128], F32, name="psmm", tag="psmm")
                nc.tensor.matmul(pswu, lhsT=ident_bf, rhs=ident_bf,
                                 start=True, stop=True)

            # w_blk = block-diag(w, w) bf16 [128, 128]
            w_f32 = consts.tile([128, NF], F32)
            nc.sync.dma_start(out=w_f32[0:NF, :], in_=w_t[:, :])
            nc.sync.dma_start(out=w_f32[NF:128, :], in_=w_t[:, :])
            w_blk = consts.tile([128, 128], BF16)
            nc.vector.memset(w_blk, 0.0)
            nc.vector.tensor_copy(w_blk[0:NF, 0:NF], w_f32[0:NF, :])
            nc.vector.tensor_copy(w_blk[NF:128, NF:128], w_f32[NF:128, :])

            # asd_blk [128, 8]: rows 0:64 cols 0:4 = [-a_src/Ds, a_dst/Dd,
            # a_src, a_dst]; rows 64:128 cols 4:8 = same (odd batch)
            asd_f32 = consts.tile([128, 4], F32)
            nc.sync.dma_start(out=asd_f32[0:NF, :], in_=asd_t[:, :])
            nc.sync.dma_start(out=asd_f32[NF:128, :], in_=asd_t[:, :])
            asd_blk = consts.tile([128, 8], BF16)
            nc.vector.memset(asd_blk, 0.0)
            nc.vector.tensor_copy(asd_blk[0:NF, 0:4], asd_f32[0:NF, :])
            nc.vector.tensor_copy(asd_blk[NF:128, 4:8], asd_f32[NF:128, :])

            iota_i32 = consts.tile([128, NBUCK], I32)
            nc.gpsimd.iota(iota_i32, pattern=[[1, NBUCK]], base=0,
                           channel_multiplier=0)
            iota_row = consts.tile([128, NBUCK], BF16)
            nc.vector.tensor_copy(iota_row, iota_i32)
            iotac_i32 = consts.tile([128, 1], I32)
            nc.gpsimd.iota(iotac_i32, pattern=[[0, 1]], base=0,
                           channel_multiplier=1)
            iota_colf = consts.tile([128, 1], F32)
            nc.vector.tensor_copy(iota_colf, iotac_i32)
            negones = consts.tile([128, 1], BF16)
            nc.vector.memset(negones, -1.0)

            # ---- load pre-transposed h: one [128, n] tile per pair,
            # partitions 0:64 = even batch, 64:128 = odd ----
            hTT = []
            for p in range(npair):
                hT2 = hpool.tile([128, n], BF16, name=f"hT2_{p}")
                nc.sync.dma_start(out=hT2[0:NF, :], in_=ht_t[2 * p, :, :])
                nc.sync.dma_start(out=hT2[NF:128, :],
                                  in_=ht_t[2 * p + 1, :, :])
                hTT.append(hT2)

            def stage1(p):
                hT2 = hTT[p]
                st = {}

                # B: T2 = tanh(w_blk.T @ hT2) [128, n] (both batches)
                T2_sb = pairbuf.tile([128, n], BF16, name="T2_sb")
                psD = psm.tile([128, NT, 8], F32, name="psD", tag="psdtr")
                for icx, (c0, cs) in enumerate(C512):
                    psB = pmm.tile([128, 512], F32, name="psmm", tag="psmm")
                    nc.tensor.matmul(
                        psB[:, 0:cs], lhsT=w_blk, rhs=hT2[:, c0:c0 + cs],
                        start=True, stop=True)
                    nc.scalar.activation(
                        T2_sb[:, c0:c0 + cs], psB[:, 0:cs], AF.Tanh)
                    # D: psD[:, jb, :] = per-batch [x_s, x_d, s, d] columns
                    # (x_s = -s/Ds, x_d = d/Dd via host-prescaled asd cols)
                    for k in range(4):
                        jb = icx * 4 + k
                        nc.tensor.matmul(
                            psD[:, jb, :],
                            lhsT=T2_sb[:, jb * 128:(jb + 1) * 128],
                            rhs=asd_blk, start=True, stop=True)

                # threshold bucket bn_i = clip(x_s + CMID): col -> row via
                # PE transpose -> DRAM roundtrip broadcast (latency hidden
                # behind the rest of stage1). No rounding: sub-bucket
                # boundary shift only.
                # tr_in[:, 0:16] = even-batch buckets, [:, 16:32] = odd
                tr_in = work.tile([128, 32], BF16, name="tr_in")
                nc.vector.tensor_scalar(
                    out=_apx(tr_in, 0, [1, NT], [NT, 2]),
                    in0=_apx(psD, 0, [8, NT], [4, 2]),
                    scalar1=CMID, scalar2=KMAX, op0=ALU.add, op1=ALU.min)
                psTr = psm.tile([32, 128], BF16, name="psTr", tag="psdtr")
                nc.tensor.transpose(psTr, tr_in, ident_bf)
                bn_row = work.tile([32, 128], BF16, name="bn_row")
                nc.scalar.copy(bn_row, psTr)
                bn_dram = drampool.tile([32, 128], BF16, name="bn_dram")
                nc.sync.dma_start(out=bn_dram, in_=bn_row)
                bdap = bn_dram[0, 0:128]
                # both batches' broadcast rows land in one [128, 2, n] tile
                bn_bc = pairbuf.tile([128, 2, n], BF16, name="bn_bc")
                for half in range(2):
                    nc.sync.dma_start(out=bn_bc[:, half, :], in_=bass.AP(
                        tensor=bdap.tensor, offset=bdap.offset + half * n,
                        ap=[[0, 128], [1, n]]))
                st["bn_bc"] = bn_bc

                # e8s / ed / ed2 columns (both batches per op)
                s_raw = _apx(psD, 2, [8, NT], [4, 2])
                d_raw = _apx(psD, 3, [8, NT], [4, 2])
                e8s2 = pairbuf.tile([128, NT, 2], F32, name="e8s2")
                nc.scalar.activation(e8s2, s_raw, AF.Exp, scale=0.8)
                # edc2 [128, NT, 4]: cols (ed_e, ed2_e, ed_o, ed2_o)
                edc2 = pairbuf.tile([128, NT, 4], BF16, name="edc2")
                nc.scalar.activation(
                    _apx(edc2, 0, [4, NT], [2, 2]), d_raw, AF.Exp)
                nc.scalar.activation(
                    _apx(edc2, 1, [4, NT], [2, 2]), d_raw, AF.Exp, scale=0.2)
                st["e8s2"] = e8s2

                # bucket(d_j): round(x_d + CMID), clip to [0, KMAX]
                rd = work.tile([128, NT, 2], F32, name="rd")
                nc.vector.tensor_scalar(
                    out=rd, in0=_apx(psD, 1, [8, NT], [4, 2]),
                    scalar1=RND + CMID, scalar2=RND,
                    op0=ALU.add, op1=ALU.subtract)
                kd2 = pairbuf.tile([128, NT, 2], BF16, name="kd2")
                nc.vector.tensor_scalar(
                    out=kd2, in0=rd, scalar1=0.0, scalar2=KMAX,
                    op0=ALU.max, op1=ALU.min)

                # A: hp_ext2[:, jb, 0:65] = [hp_e | 1], [66:131] = [hp_o | 1]
                hp_ext2 = pairbuf.tile([128, NT, 132], BF16, name="hp_ext2")
                nc.vector.memset(_apx(hp_ext2, NF, [132, NT], [NF + 2, 2]),
                                 1.0)
                for (j0, js) in _chunks(NT, 4):
                    psA = pmm.tile([128, 4, 128], F32, name="psmm",
                                   tag="psmm")
                    for k in range(js):
                        jb = j0 + k
                        nc.tensor.matmul(
                            psA[:, k, :],
                            lhsT=hT2[:, jb * 128:(jb + 1) * 128],
                            rhs=w_blk, start=True, stop=True)
                    nc.scalar.copy(
                        hp_ext2[:, j0:j0 + js, 0:NF], psA[:, 0:js, 0:NF])
                    nc.scalar.copy(
                        hp_ext2[:, j0:j0 + js, NF + 2:NF * 2 + 2],
                        psA[:, 0:js, NF:128])

                # values: edhp_b = [ed*hp_ext | ed2*hp_ext], one op per batch
                for half, nm in ((0, "edhp_e"), (1, "edhp_o")):
                    edhp = pairbuf.tile([128, NT, NW], BF16, name=nm)
                    nc.vector.tensor_tensor(
                        out=_apx(edhp, 0, [NW, NT], [65, 2], [1, 65]),
                        in0=_apx(hp_ext2, half * (NF + 2),
                                 [132, NT], [0, 2], [1, 65]),
                        in1=_apx(edc2, half * 2, [4, NT], [1, 2], [0, 65]),
                        op=ALU.mult)
                    st[nm] = edhp

                # masks: onehot[j, jb, b, k] = (kd[j,jb,b] == k), one op
                onehot2 = pairbuf.tile([128, NT, 2, NBUCK], BF16,
                                       name="onehot2")
                iap = iota_row[:, :]
                nc.vector.tensor_tensor(
                    out=onehot2,
                    in0=_apx(kd2, 0, [2, NT], [1, 2], [0, NBUCK]),
                    in1=bass.AP(tensor=iap.tensor, offset=iap.offset,
                                ap=[list(iap.ap[0]), [0, NT], [0, 2],
                                    [1, NBUCK]]),
                    op=ALU.is_equal)
                st["onehot2"] = onehot2
                return st

            def stageF(st, half):
                # scatter into combined table, then -Tot2 into row 127.
                sfx = "_e" if half == 0 else "_o"
                onehot2 = st["onehot2"]
                edhp = st["edhp" + sfx]
                psT12 = pscat.tile([128, 256], F32, name="psT12")
                for jb in range(NT):
                    nc.tensor.matmul(
                        psT12[:, 0:NW], lhsT=onehot2[:, jb, half, :],
                        rhs=edhp[:, jb, :],
                        start=(jb == 0), stop=(jb == NT - 1))
                T12_sb = pairbuf.tile([128, NW], BF16, name="T12" + sfx)
                nc.scalar.copy(T12_sb, psT12[:, 0:NW])
                nc.tensor.matmul(
                    psT12[0:1, 130:195], lhsT=negones[0:127, 0:1],
                    rhs=T12_sb[0:127, 65:130], start=True, stop=True,
                    skip_group_check=True)
                totrow = work.tile([1, 65], BF16, name="totrow" + sfx)
                nc.scalar.copy(totrow, psT12[0:1, 130:195])
                nc.sync.dma_start(out=T12_sb[127:128, 65:130], in_=totrow)
                st["T12" + sfx] = T12_sb

            def stageG(st, p, half):
                sfx = "_e" if half == 0 else "_o"
                b = 2 * p + half
                bn_bc = st["bn_bc"]
                T12_sb = st["T12" + sfx]
                e8s2 = st["e8s2"]
                # step mask: hge[k,i] = (t_i <= k)
                hge = pairbuf.tile([128, n], BF16, name="hge" + sfx)
                nc.vector.tensor_scalar(
                    out=hge, in0=bn_bc[:, half, :], scalar1=iota_colf,
                    scalar2=None, op0=ALU.is_le)
                o_full = outp.tile([128, NT, NF], BF16, name="o_full" + sfx)
                # G-matmul waves -> scalar-engine copy to SBUF; the whole
                # batch then combines in 3 wide DVE ops instead of 12
                gsb = work.tile([128, NT, NW], F32, name="gsb")
                for wv, w0 in enumerate(range(0, NT, WAVE)):
                    ws = min(WAVE, NT - w0)
                    pool_w = pGa if wv % 2 == 0 else pGb
                    psG = pool_w.tile([128, WAVE, 256], F32,
                                      name=f"psG{'ab'[wv % 2]}")
                    for k in range(ws):
                        it = w0 + k
                        nc.tensor.matmul(
                            psG[:, k, 0:NW],
                            lhsT=hge[:, it * 128:(it + 1) * 128],
                            rhs=T12_sb, start=True, stop=True)
                    nc.scalar.copy(gsb[:, w0:w0 + ws, :],
                                   psG[:, 0:ws, 0:NW])
                # tmp = e8s*G1 ; numn = (G2-Tot2) - tmp = -num
                tmp = work.tile([128, NT, 65], F32, name="tmp")
                e8b = e8s2[:, :, :]
                e8ap = bass.AP(
                    tensor=e8b.tensor, offset=e8b.offset + half,
                    ap=[list(e8b.ap[0]), [2, NT], [0, 65]])
                nc.vector.tensor_tensor(
                    out=tmp, in0=_apx(gsb, 0, [NW, NT], [1, 65]),
                    in1=e8ap, op=ALU.mult)
                numn = work.tile([128, NT, 65], F32, name="numn")
                nc.vector.tensor_tensor(
                    out=numn, in0=_apx(gsb, 65, [NW, NT], [1, 65]),
                    in1=tmp, op=ALU.subtract)
                r = work.tile([128, NT], F32, name="r")
                nc.vector.reciprocal(r, numn[:, :, 64:65])
                # out = (-num)*(-1/den); alternate gpsimd / vector per batch
                eng = nc.gpsimd if b % 2 == 0 else nc.vector
                eng.tensor_tensor(
                    out=o_full, in0=numn[:, :, 0:64],
                    in1=_rep0(r, NF), op=ALU.mult)
                oap = o_t[b, :, :]
                nc.sync.dma_start(
                    out=bass.AP(tensor=oap.tensor, offset=oap.offset,
                                ap=[[NT * NF, 128], [NF, NT], [1, NF]]),
                    in_=o_full)

            # software pipeline: all scatters before all gathers so the PE
            # in-order stream never stalls on a roundtrip DMA
            st0 = stage1(0)
            stageF(st0, 0)
            stageF(st0, 1)
            st1 = stage1(1)
            stageF(st1, 0)
            stageF(st1, 1)
            stageG(st0, 0, 0)
            stageG(st0, 0, 1)
            stageG(st1, 1, 0)
            stageG(st1, 1, 1)

    nc.compile()
    return nc


_CACHE = {}
_last_results = None


def _get_nc(n=2048, nb=NB):
    key = (n, nb)
    if key not in _CACHE:
        _CACHE[key] = build_gat_module(n, nb)
    return _CACHE[key]


def kernel(h, adj, w, a_src, a_dst, bias_p):
    global _last_results
    h = np.asarray(h, dtype=np.float32)
    w = np.asarray(w, dtype=np.float32)
    a_src = np.asarray(a_src, dtype=np.float32)
    a_dst = np.asarray(a_dst, dtype=np.float32)
    bias_p = np.asarray(bias_p, dtype=np.float32)
    nb, n, _ = h.shape
    NT = n // 128

    ht = np.ascontiguousarray(
        np.transpose(h, (0, 2, 1))).astype(ml_dtypes.bfloat16)

    # adaptive bucket scales: max|s|, max|d| per head (BLAS, cheap)
    hf = h.reshape(-1, h.shape[-1])
    nc = _get_nc(n, nb)
    in_maps = []
    for c in range(NH):
        th = np.tanh(hf @ w[c])
        s = th @ a_src[c, :, 0]
        d = th @ a_dst[c, :, 0]
        # one SHARED bucket scale: the mask compares s- and d-bucket
        # indices, which is only order-consistent if both use the same
        # delta
        dlt = max(float(np.abs(s).max()), float(np.abs(d).max()),
                  1e-6) / 62.0
        asd = np.stack([-a_src[c, :, 0] / dlt, a_dst[c, :, 0] / dlt,
                        a_src[c, :, 0], a_dst[c, :, 0]],
                       axis=1).astype(np.float32)
        in_maps.append({
            "ht": ht,
            "w1": np.ascontiguousarray(w[c]),
            "asd": np.ascontiguousarray(asd),
        })
    res = run_bass_kernel_spmd(nc, in_maps, core_ids=list(range(NH)))
    _last_results = res
    out = np.empty((nb, NH, n, NF), np.float32)
    for c in range(NH):
        # device layout [nb, 128, NT*NF] bf16 -> [nb, n, NF] f32
        dev = res.results[c]["out"].astype(np.float32)
        out[:, c] = dev.reshape(nb, 128, NT, NF).transpose(
            0, 2, 1, 3).reshape(nb, n, NF)
    # bias applied on host: out = attn@hp + bias (exact)
    out += bias_p[None, None, None, :]
    return out
